# revision 9
# baseline (speedup 1.0000x reference)
"""CrossScaleAttention Trainium2 kernel.

Data-parallel over batch: 16 samples / 8 cores = 2 samples per core.

Algebraic restructuring (all exact up to fp reassociation):
  - 1x1 convs commute with nearest 2x upsample -> whole main branch runs at 32x32.
  - W_align folded into main qkv:  W_qkv_m @ W_align.
  - pos embeds / rel_pos / biases folded into GEMM bias vectors.
  - attention score via quadratic form:  score = x^T A x + u.x (+const dropped:
    softmax shift-invariant), A = Wq^T Wk / sqrt(C), so q/k are never computed.
  - proj GEMMs folded into fuse:  W_f* = W_fuse_half @ W_proj (+1/2 for the
    upsampled-softmax duplication on the main branch).
  - attn applied post-GEMM:  W@(attn*v) == attn*(W@v).
Matmuls run in float32r (TF32-like, ~1e-4 rel err, 4x fp32 rate at N>=512).
"""
import sys
sys.path.insert(0, '/opt/trn_rl_repo')
import numpy as np

B, CS, CM = 16, 256, 512
HS = WS = 64
HM = WM = 32
NPX_S = HS * WS          # 4096
NPX_M = HM * WM          # 1024
NCORES = 8
BPC = B // NCORES        # 2 samples per core
CHUNK = 1024             # small-branch pixel chunk (16 h-rows)
NCHUNK = NPX_S // CHUNK  # 4

_prog = None  # (nc, input_names) cache


def _build_program():
    import concourse.bacc as bacc
    import concourse.mybir as mybir
    from concourse.tile import TileContext

    f32, f32r = mybir.dt.float32, mybir.dt.float32r
    Act = mybir.ActivationFunctionType
    Alu = mybir.AluOpType

    nc = bacc.Bacc(None, target_bir_lowering=False)

    xs_d = nc.dram_tensor("xs", [BPC, CS, NPX_S], f32r, kind="ExternalInput")
    xm_d = nc.dram_tensor("xm", [BPC, CM, NPX_M], f32r, kind="ExternalInput")
    AsT_d = nc.dram_tensor("AsT", [CS, CS], f32r, kind="ExternalInput")
    WvsT_d = nc.dram_tensor("WvsT", [CS, CS], f32r, kind="ExternalInput")
    WfsT_d = nc.dram_tensor("WfsT", [CS, CS], f32r, kind="ExternalInput")
    AmT_d = nc.dram_tensor("AmT", [CM, CM], f32r, kind="ExternalInput")
    WvmT_d = nc.dram_tensor("WvmT", [CM, CS], f32r, kind="ExternalInput")
    WfmT_d = nc.dram_tensor("WfmT", [CS, CS], f32r, kind="ExternalInput")
    us_d = nc.dram_tensor("us", [128, 2], f32, kind="ExternalInput")
    cvs_d = nc.dram_tensor("cvs", [128, 2], f32, kind="ExternalInput")
    um_d = nc.dram_tensor("um", [128, 4], f32, kind="ExternalInput")
    cvm_d = nc.dram_tensor("cvm", [128, 2], f32, kind="ExternalInput")
    bout_d = nc.dram_tensor("bout", [128, 2], f32, kind="ExternalInput")
    out_d = nc.dram_tensor("out", [BPC, CS, NPX_S], f32, kind="ExternalOutput")

    with TileContext(nc) as tc:
        with (
            tc.tile_pool(name="wp", bufs=1) as wp,
            tc.tile_pool(name="mp", bufs=1) as mp,   # main-branch tiles (explicit bufs via tags)
            tc.tile_pool(name="sp", bufs=1) as sp,   # small-branch tiles
            tc.tile_pool(name="ps_y", bufs=3, space="PSUM") as ps_y,
            tc.tile_pool(name="ps_v", bufs=2, space="PSUM") as ps_v,
            tc.tile_pool(name="ps_f", bufs=2, space="PSUM") as ps_f,
            tc.tile_pool(name="ps_s", bufs=1, space="PSUM") as ps_s,
        ):
            # ---- resident weights ----
            def wtile(dram, rows, cols, name):
                ts = []
                for k in range(rows // 128):
                    t = wp.tile([128, cols], f32r, tag=f"{name}{k}")
                    nc.sync.dma_start(out=t[:], in_=dram[k * 128:(k + 1) * 128, :])
                    ts.append(t)
                return ts

            xm_all = []
            for b in range(BPC):
                xm_b = []
                for k in range(4):
                    t = mp.tile([128, NPX_M], f32r, tag="xm", bufs=6, name=f"xm{b}_{k}")
                    nc.sync.dma_start(out=t[:], in_=xm_d[b, k * 128:(k + 1) * 128, :])
                    xm_b.append(t)
                xm_all.append(xm_b)

            AmT = wtile(AmT_d, CM, CM, "AmT")
            WvmT = wtile(WvmT_d, CM, CS, "WvmT")
            WfmT = wtile(WfmT_d, CS, CS, "WfmT")
            AsT = wtile(AsT_d, CS, CS, "AsT")
            WvsT = wtile(WvsT_d, CS, CS, "WvsT")
            WfsT = wtile(WfsT_d, CS, CS, "WfsT")

            def vec(dram, cols, name):
                t = wp.tile([128, cols], f32, tag=name)
                nc.sync.dma_start(out=t[:], in_=dram[:])
                return t

            us = vec(us_d, 2, "us")
            cvs = vec(cvs_d, 2, "cvs")
            um = vec(um_d, 4, "um")
            cvm = vec(cvm_d, 2, "cvm")
            bout = vec(bout_d, 2, "bout")

            ones32 = wp.tile([128, 1], f32, tag="ones32")
            nc.vector.memset(ones32[:], 1.0)
            ones = wp.tile([128, 1], f32r, tag="ones")
            nc.vector.tensor_copy(ones[:], ones32[:])

            smalls = []
            for b in range(BPC):
                # ================= main branch (at 32x32) =================
                xm = xm_all[b]

                # y' = A_m @ x + u_m ; t_m = x * y'
                tm = [mp.tile([128, NPX_M], f32r, tag=f"tm{m}", bufs=2, name=f"tm{m}") for m in range(4)]
                for n in range(2):
                    for m in range(4):
                        py = ps_y.tile([128, 512], f32, tag="y")
                        for k in range(4):
                            nc.tensor.matmul(py[:], AmT[k][:, m * 128:(m + 1) * 128],
                                             xm[k][:, n * 512:(n + 1) * 512],
                                             start=(k == 0), stop=(k == 3))
                        nc.vector.scalar_tensor_tensor(
                            tm[m][:, n * 512:(n + 1) * 512], py[:], um[:, m:m + 1],
                            xm[m][:, n * 512:(n + 1) * 512], op0=Alu.add, op1=Alu.mult)

                # v_m = Wvm @ x + cvm  (f32r)
                vm = [mp.tile([128, NPX_M], f32r, tag=f"vm{m}", bufs=1, name=f"vm{m}") for m in range(2)]
                for n in range(2):
                    for m in range(2):
                        pv = ps_v.tile([128, 512], f32, tag="v")
                        for k in range(4):
                            nc.tensor.matmul(pv[:], WvmT[k][:, m * 128:(m + 1) * 128],
                                             xm[k][:, n * 512:(n + 1) * 512],
                                             start=(k == 0), stop=(k == 3))
                        nc.scalar.activation(vm[m][:, n * 512:(n + 1) * 512], pv[:],
                                             Act.Identity, bias=cvm[:, m:m + 1], scale=1.0)

                # score = sum_c t_m -> [1, 1024]
                smf = mp.tile([1, NPX_M], f32, tag="smf", bufs=1)
                for n in range(2):
                    pscr = ps_s.tile([1, 512], f32, tag="s")
                    for k in range(4):
                        nc.tensor.matmul(pscr[:], ones[:], tm[k][:, n * 512:(n + 1) * 512],
                                         start=(k == 0), stop=(k == 3))
                    nc.scalar.activation(smf[:, n * 512:(n + 1) * 512], pscr[:], Act.Copy)

                # softmax over w (32-wide rows)
                shw = mp.tile([32, 32], f32, tag="shw", bufs=2)
                nc.sync.dma_start(out=shw[:], in_=smf[:])
                nmax = mp.tile([32, 1], f32, tag="nmax", bufs=2)
                nc.vector.tensor_reduce(nmax[:], shw[:], axis=mybir.AxisListType.X,
                                        op=Alu.max, negate=True)
                ex = mp.tile([32, 32], f32, tag="ex", bufs=2)
                esum = mp.tile([32, 1], f32, tag="esum", bufs=2)
                nc.scalar.activation(ex[:], shw[:], Act.Exp, bias=nmax[:], scale=1.0,
                                     accum_out=esum[:])
                rec = mp.tile([32, 1], f32, tag="rec", bufs=2)
                nc.vector.reciprocal(rec[:], esum[:])
                attnm = mp.tile([32, 32], f32, tag="attnm", bufs=2)
                nc.scalar.activation(attnm[:], ex[:], Act.Copy, scale=rec[:])
                amf = mp.tile([1, NPX_M], f32, tag="amf", bufs=1)
                nc.sync.dma_start(out=amf[:], in_=attnm[:])
                amb = mp.tile([128, NPX_M], f32, tag="amb", bufs=2)
                nc.gpsimd.partition_broadcast(amb[:], amf[:])

                # fm = attn_m * (W_fm @ v_m)   [256,1024] f32, resident for small branch
                fm = [mp.tile([128, NPX_M], f32, tag=f"fm{m}", bufs=2, name=f"fm{m}") for m in range(2)]
                for n in range(2):
                    for m in range(2):
                        pf = ps_f.tile([128, 512], f32, tag="f")
                        for k in range(2):
                            nc.tensor.matmul(pf[:], WfmT[k][:, m * 128:(m + 1) * 128],
                                             vm[k][:, n * 512:(n + 1) * 512],
                                             start=(k == 0), stop=(k == 1))
                        nc.vector.tensor_mul(fm[m][:, n * 512:(n + 1) * 512], pf[:],
                                             amb[:, n * 512:(n + 1) * 512])

                smalls.append((b, fm))

            # ================= small branches, chunk-interleaved across samples ========
            for c in range(NCHUNK):
                for b, fm in smalls:
                    px0 = c * CHUNK
                    xs = []
                    for k in range(2):
                        t = sp.tile([128, CHUNK], f32r, tag="xs", bufs=4)
                        nc.sync.dma_start(out=t[:],
                                          in_=xs_d[b, k * 128:(k + 1) * 128, px0:px0 + CHUNK])
                        xs.append(t)

                    ts_ = [sp.tile([128, CHUNK], f32r, tag=f"ts{m}", bufs=2, name=f"ts{m}") for m in range(2)]
                    for n in range(2):
                        for m in range(2):
                            py = ps_y.tile([128, 512], f32, tag="y")
                            for k in range(2):
                                nc.tensor.matmul(py[:], AsT[k][:, m * 128:(m + 1) * 128],
                                                 xs[k][:, n * 512:(n + 1) * 512],
                                                 start=(k == 0), stop=(k == 1))
                            nc.vector.scalar_tensor_tensor(
                                ts_[m][:, n * 512:(n + 1) * 512], py[:], us[:, m:m + 1],
                                xs[m][:, n * 512:(n + 1) * 512], op0=Alu.add, op1=Alu.mult)

                    vs = [sp.tile([128, CHUNK], f32r, tag=f"vs{m}", bufs=2, name=f"vs{m}") for m in range(2)]
                    for n in range(2):
                        for m in range(2):
                            pv = ps_v.tile([128, 512], f32, tag="v")
                            for k in range(2):
                                nc.tensor.matmul(pv[:], WvsT[k][:, m * 128:(m + 1) * 128],
                                                 xs[k][:, n * 512:(n + 1) * 512],
                                                 start=(k == 0), stop=(k == 1))
                            nc.scalar.activation(vs[m][:, n * 512:(n + 1) * 512], pv[:],
                                                 Act.Identity, bias=cvs[:, m:m + 1], scale=1.0)

                    sf = sp.tile([1, CHUNK], f32, tag="sf", bufs=2)
                    for n in range(2):
                        pscr = ps_s.tile([1, 512], f32, tag="s")
                        for k in range(2):
                            nc.tensor.matmul(pscr[:], ones[:],
                                             ts_[k][:, n * 512:(n + 1) * 512],
                                             start=(k == 0), stop=(k == 1))
                        nc.scalar.activation(sf[:, n * 512:(n + 1) * 512], pscr[:], Act.Copy)

                    shw_s = sp.tile([16, 64], f32, tag="shw_s", bufs=2)
                    nc.sync.dma_start(out=shw_s[:], in_=sf[:])
                    nmax_s = sp.tile([16, 1], f32, tag="nmax_s", bufs=2)
                    nc.vector.tensor_reduce(nmax_s[:], shw_s[:], axis=mybir.AxisListType.X,
                                            op=Alu.max, negate=True)
                    ex_s = sp.tile([16, 64], f32, tag="ex_s", bufs=2)
                    esum_s = sp.tile([16, 1], f32, tag="esum_s", bufs=2)
                    nc.scalar.activation(ex_s[:], shw_s[:], Act.Exp, bias=nmax_s[:],
                                         scale=1.0, accum_out=esum_s[:])
                    rec_s = sp.tile([16, 1], f32, tag="rec_s", bufs=2)
                    nc.vector.reciprocal(rec_s[:], esum_s[:])
                    attn_s = sp.tile([16, 64], f32, tag="attn_s", bufs=2)
                    nc.scalar.activation(attn_s[:], ex_s[:], Act.Copy, scale=rec_s[:])
                    af = sp.tile([1, CHUNK], f32, tag="af", bufs=2)
                    nc.sync.dma_start(out=af[:], in_=attn_s[:])
                    ab = sp.tile([128, CHUNK], f32, tag="ab", bufs=2)
                    nc.gpsimd.partition_broadcast(ab[:], af[:])

                    fs = [sp.tile([128, CHUNK], f32, tag=f"fs{m}", bufs=2, name=f"fs{m}") for m in range(2)]
                    for n in range(2):
                        for m in range(2):
                            pf = ps_f.tile([128, 512], f32, tag="f")
                            for k in range(2):
                                nc.tensor.matmul(pf[:], WfsT[k][:, m * 128:(m + 1) * 128],
                                                 vs[k][:, n * 512:(n + 1) * 512],
                                                 start=(k == 0), stop=(k == 1))
                            nc.vector.tensor_mul(fs[m][:, n * 512:(n + 1) * 512], pf[:],
                                                 ab[:, n * 512:(n + 1) * 512])

                    # out = fs + bout + upsample(fm)   (4 strided adds per m)
                    h2a, h2b = c * 8, c * 8 + 8
                    for m in range(2):
                        f5 = fs[m][:].rearrange("p (h2 dh wj dw) -> p h2 dh wj dw",
                                                h2=8, dh=2, wj=32, dw=2)
                        fm3 = fm[m][:].rearrange("p (h2 wj) -> p h2 wj", wj=32)
                        for dh in range(2):
                            for dw in range(2):
                                nc.vector.scalar_tensor_tensor(
                                    f5[:, :, dh, :, dw], f5[:, :, dh, :, dw],
                                    bout[:, m:m + 1], fm3[:, h2a:h2b, :],
                                    op0=Alu.add, op1=Alu.add)
                        nc.sync.dma_start(
                            out=out_d[b, m * 128:(m + 1) * 128, px0:px0 + CHUNK],
                            in_=fs[m][:])

    nc.compile()
    return nc


def _prep_weights(W_align, b_align, pos_embed_main, pos_embed_small,
                  W_qkv_s, b_qkv_s, W_proj_s, b_proj_s, rel_pos_s,
                  W_qkv_m, b_qkv_m, W_proj_m, b_proj_m, rel_pos_m,
                  W_fuse, b_fuse):
    d = np.float64
    W_align, b_align = W_align.astype(d), b_align.astype(d)
    pos_s = pos_embed_small.reshape(-1).astype(d)
    pos_m = pos_embed_main.reshape(-1).astype(d)
    rel_s = rel_pos_s.reshape(-1).astype(d)
    rel_m = rel_pos_m.reshape(-1).astype(d)
    W_qkv_s, b_qkv_s = W_qkv_s.astype(d), b_qkv_s.astype(d)
    W_qkv_m, b_qkv_m = W_qkv_m.astype(d), b_qkv_m.astype(d)
    W_proj_s, b_proj_s = W_proj_s.astype(d), b_proj_s.astype(d)
    W_proj_m, b_proj_m = W_proj_m.astype(d), b_proj_m.astype(d)
    W_fuse, b_fuse = W_fuse.astype(d), b_fuse.astype(d)
    scale = 1.0 / np.sqrt(np.float64(CS))

    # small branch
    Wq, Wk, Wv = W_qkv_s[:CS], W_qkv_s[CS:2 * CS], W_qkv_s[2 * CS:]
    bq, bk, bv = b_qkv_s[:CS], b_qkv_s[CS:2 * CS], b_qkv_s[2 * CS:]
    cq = Wq @ pos_s + bq + rel_s
    ck = Wk @ pos_s + bk + rel_s
    cv_s = Wv @ pos_s + bv
    A_s = (Wq.T @ Wk) * scale
    u_s = (Wk.T @ cq + Wq.T @ ck) * scale
    Wv_s = Wv

    # main branch (W_align folded; runs at 32x32)
    cbase = b_align + pos_m
    Wqm = W_qkv_m[:CS] @ W_align
    Wkm = W_qkv_m[CS:2 * CS] @ W_align
    Wvm = W_qkv_m[2 * CS:] @ W_align
    cqm = W_qkv_m[:CS] @ cbase + b_qkv_m[:CS] + rel_m
    ckm = W_qkv_m[CS:2 * CS] @ cbase + b_qkv_m[CS:2 * CS] + rel_m
    cv_m = W_qkv_m[2 * CS:] @ cbase + b_qkv_m[2 * CS:]
    A_m = (Wqm.T @ Wkm) * scale
    u_m = (Wkm.T @ cqm + Wqm.T @ ckm) * scale

    # fuse folding (1/2 on main: upsampled softmax over 64 = low-res softmax / 2)
    W_fs = W_fuse[:, :CS] @ W_proj_s
    W_fm = (W_fuse[:, CS:] @ W_proj_m) * 0.5
    b_out = b_fuse + W_fuse[:, :CS] @ b_proj_s + W_fuse[:, CS:] @ b_proj_m

    f = np.float32
    def colvec(v, chunks):
        return np.ascontiguousarray(v.reshape(chunks, 128).T.astype(f))
    return {
        "AsT": np.ascontiguousarray(A_s.T.astype(f)),
        "WvsT": np.ascontiguousarray(Wv_s.T.astype(f)),
        "WfsT": np.ascontiguousarray(W_fs.T.astype(f)),
        "AmT": np.ascontiguousarray(A_m.T.astype(f)),
        "WvmT": np.ascontiguousarray(Wvm.T.astype(f)),
        "WfmT": np.ascontiguousarray(W_fm.T.astype(f)),
        "us": colvec(u_s, 2), "cvs": colvec(cv_s, 2),
        "um": colvec(u_m, 4), "cvm": colvec(cv_m, 2),
        "bout": colvec(b_out, 2),
    }


def kernel(**inputs):
    global _prog
    from concourse.bass_utils import run_bass_kernel_spmd

    small = np.ascontiguousarray(np.asarray(inputs["small_feat"], dtype=np.float32))
    main = np.ascontiguousarray(np.asarray(inputs["main_feat"], dtype=np.float32))
    w = _prep_weights(**{k: np.asarray(v) for k, v in inputs.items()
                         if k not in ("small_feat", "main_feat")})

    if _prog is None:
        _prog = _build_program()
    nc = _prog

    in_maps = []
    for c in range(NCORES):
        m = dict(w)
        m["xs"] = small[c * BPC:(c + 1) * BPC].reshape(BPC, CS, NPX_S)
        m["xm"] = main[c * BPC:(c + 1) * BPC].reshape(BPC, CM, NPX_M)
        in_maps.append(m)

    res = run_bass_kernel_spmd(nc, in_maps, list(range(NCORES)))
    out = np.concatenate([r["out"] for r in res.results], axis=0)
    return out.reshape(B, CS, HS, WS)
